# revision 61
# baseline (speedup 1.0000x reference)
"""Trainium2 Bass kernel for the Koopman DEINA model.

Computation (per reference):
  encoder: h1 = relu(x W0^T + b0); h2 = relu(h1 W1^T + b1); g = h2 W2^T
  y2s      = concat(xs[:,1:], g[:,1:])                       [B,127,80]
  y2s_pred = scan: p_{n+1} = K p_n + Bw u_n, p_0 = [x_0; g_0], output p_1..p_127

Strategy (8 NeuronCores, data-parallel over batch, 256 batch elems/core):
  - Encoder in bf16 with fp32 psum; tokens processed t-major (token = t*256+b),
    x PE-transposed in 512-token tiles (4 groups x 32 cols, 16 zero-weight pad).
  - PE pipeline continuity is the design driver: matmuls are emitted so the
    array never waits on a drain (measured: back-to-back matmuls stream at
    ~0.42ns/row with weight loads hidden; dependency breaks cost ~2x).
  - y2s x-part: two whole-tensor DRAM->DRAM DMAs (no SBUF hop, no copies).
  - y2s g-part: f32 PE transposes into PSUM, DMA'd straight from PSUM.
  - pred: chunked closed form (S=8), phase-C outputs DMA'd straight from PSUM.
  - u transposes and the 15-step boundary chain are interleaved into the
    encoder superblock loop so their latency hides under encoder matmuls.
"""
import os
import numpy as np

DIM = 16
H1 = H2 = 256
G = 64
L = 80          # DIM + G
T = 128
NB = 2048       # full batch
NCORES = 8
BC = NB // NCORES   # 256 batch elems per core
S = 8           # recurrence chunk size
NCH = T // S    # 16 chunks
NTOK = BC * T   # 32768 tokens per core
NSB = NTOK // 2048   # 16 superblocks of 2048 tokens (= 8 t-steps x 256 b)

# f32r const tile column offsets (recurrence)
OFF_RA = 0        # [80, 640]   R_A[k,(s-1)*80+l] = (K^s)[l,k]
OFF_RX = 640      # [16, 640]   R_X[d,(s-1)*80+l] = (K^s)[l,d]
OFF_RG = 1280     # [128, 1280] R_G[kp, kc*640+(s-1)*80+l] = (K^s[:,16:] @ W2)[l, kc*128+kp]
OFF_W0R = 2560    # [16, 256]   exact W0T (f32r) for the h2_0 recompute
OFF_W1R = 2816    # [128, 512]  exact W1T (f32r)
CW_COLS = 3328
# bf16 const tile column offsets (encoder)
OFF_W0 = 0        # [128, 256]  W0T on 4x32-row groups (rows 0:16 of each)
OFF_W0S = 256     # [128, 256]  shifted variant (rows 16:32 of each group)
OFF_W1 = 512      # [128, 512]  W1T, kc blocks of 256 cols
OFF_W2 = 1024     # [128, 128]  W2T, kc blocks of 64 cols
OFF_IDH = 1152    # [128, 128]  bf16 identity (host-provided)
OFF_RUH = 1280    # [128, 640]  R_U[j*16+d,(s-1)*80+l] = (K^{s-1-j} Bw)[l,d], j<s (bf16)
CWH_COLS = 1920

_BUILT = None


def _precompute_consts(W0, b0, W1, b1, W2, Bw, K):
    """Host-side weight folding: CW [128,CW_COLS] f32(r), CWH [128,CWH_COLS] bf16,
    CB [128,4] f32."""
    import ml_dtypes
    W0 = np.asarray(W0, np.float64)
    W1 = np.asarray(W1, np.float64)
    W2 = np.asarray(W2, np.float64)
    Bw = np.asarray(Bw, np.float64)
    K = np.asarray(K, np.float64)

    CWH = np.zeros((128, CWH_COLS), np.float64)
    for g in range(4):
        CWH[g * 32:g * 32 + 16, OFF_W0:OFF_W0 + 256] = W0.T
        CWH[g * 32 + 16:g * 32 + 32, OFF_W0S:OFF_W0S + 256] = W0.T
    for kc in range(2):
        CWH[:, OFF_W1 + kc * 256:OFF_W1 + (kc + 1) * 256] = W1[:, kc * 128:(kc + 1) * 128].T
        CWH[:, OFF_W2 + kc * 64:OFF_W2 + (kc + 1) * 64] = W2[:, kc * 128:(kc + 1) * 128].T

    CW = np.zeros((128, CW_COLS), np.float64)
    CW[0:16, OFF_W0R:OFF_W0R + 256] = W0.T
    for kc in range(2):
        CW[:, OFF_W1R + kc * 256:OFF_W1R + (kc + 1) * 256] = W1[:, kc * 128:(kc + 1) * 128].T
    A = [np.eye(L)]
    for _ in range(S):
        A.append(K @ A[-1])
    Dm = [Bw]
    for _ in range(S - 1):
        Dm.append(K @ Dm[-1])
    for s in range(1, S + 1):
        col = (s - 1) * L
        CW[0:L, OFF_RA + col:OFF_RA + col + L] = A[s].T
        CW[0:16, OFF_RX + col:OFF_RX + col + L] = A[s][:, :16].T
        AsG = A[s][:, 16:] @ W2          # [80, 256]
        for kc in range(2):
            CW[:, OFF_RG + kc * 640 + col:OFF_RG + kc * 640 + col + L] = \
                AsG[:, kc * 128:(kc + 1) * 128].T
        for j in range(s):
            CWH[j * 16:(j + 1) * 16, OFF_RUH + col:OFF_RUH + col + L] = Dm[s - 1 - j].T

    CWH[:, OFF_IDH:OFF_IDH + 128] = np.eye(128)

    CB = np.zeros((128, 4), np.float64)
    CB[:, 0] = np.asarray(b0)[0:128]
    CB[:, 1] = np.asarray(b0)[128:256]
    CB[:, 2] = np.asarray(b1)[0:128]
    CB[:, 3] = np.asarray(b1)[128:256]
    return (CW.astype(np.float32), CWH.astype(ml_dtypes.bfloat16),
            CB.astype(np.float32), np.eye(128, dtype=np.float32))


def _build():
    import concourse.bass as bass
    import concourse.bacc as bacc
    import concourse.mybir as mybir
    import concourse.tile as tile
    
    f32 = mybir.dt.float32
    f32r = mybir.dt.float32r
    bf16 = mybir.dt.bfloat16
    AF = mybir.ActivationFunctionType
    ALU = mybir.AluOpType
    P = 128

    nc = bacc.Bacc(None, target_bir_lowering=False, debug=False)

    xs_d = nc.dram_tensor("xs", [BC, T, DIM], f32, kind="ExternalInput")
    us_d = nc.dram_tensor("us", [BC, T, DIM], f32, kind="ExternalInput")
    cw_d = nc.dram_tensor("CW", [P, CW_COLS], f32r, kind="ExternalInput")
    cwh_d = nc.dram_tensor("CWH", [P, CWH_COLS], bf16, kind="ExternalInput")
    cb_d = nc.dram_tensor("CB", [P, 4], f32, kind="ExternalInput")
    ci_d = nc.dram_tensor("CI", [P, P], f32, kind="ExternalInput")
    y2s_d = nc.dram_tensor("y2s", [BC, T - 1, L], f32, kind="ExternalOutput")
    pred_d = nc.dram_tensor("y2s_pred", [BC, T - 1, L], f32, kind="ExternalOutput")
    dbg_d = None
    if os.environ.get("DEINA_DBG_UT"):
        dbg_d = nc.dram_tensor("dbg_uT", [P, NCH * BC], bf16, kind="ExternalOutput")

    xs_ap = xs_d.ap()
    us2d = us_d.ap().rearrange("b t d -> b (t d)")     # [256, 2048]

    with tile.TileContext(nc) as tc:
        with (
            tc.tile_pool(name="consts", bufs=1) as consts,
            tc.tile_pool(name="store", bufs=1) as store,
            tc.tile_pool(name="ld", bufs=4) as ld,
            tc.tile_pool(name="aupool", bufs=8) as aupool,
            tc.tile_pool(name="xT", bufs=3) as xTp,
            tc.tile_pool(name="h1sb", bufs=2) as h1sbp,
            tc.tile_pool(name="h2sb", bufs=2) as h2sbp,
            tc.tile_pool(name="osb", bufs=8) as osbp,
        ):
            CI = consts.tile([P, P], f32)
            nc.sync.dma_start(CI[:], ci_d[:])
            CB = consts.tile([P, 4], f32)
            nc.sync.dma_start(CB[:], cb_d[:])
            CWH = consts.tile([P, CWH_COLS], bf16)
            nc.sync.dma_start(CWH[:], cwh_d[:])
            CW = consts.tile([P, CW_COLS], f32r)

            uT = store.tile([P, NCH * BC], bf16)       # [(j,d), c*256 + b]
            p8 = store.tile([80, NCH * BC], f32r)      # block c = p_{8c} (c>=1)
            x0T = store.tile([16, BC], f32r)
            h10 = store.tile([P, 2, BC], f32r)
            h2_0 = store.tile([P, 2, BC], f32r)


            def pool_copy(dst, src, alt=None):
                # psum-sourced copy: Pool/GpSimd cannot read PSUM, so these
                # go to DVE (vector) or ACT (scalar)
                e = alt or nc.vector
                if e is nc.scalar:
                    e.copy(dst, src)
                else:
                    e.tensor_copy(dst, src)

            def cwr(r0, r1, c0, c1):
                return CW[r0:r1, c0:c1]

            def cwh(r0, r1, c0, c1):
                return CWH[r0:r1, c0:c1]

            identh = CWH[:, OFF_IDH:OFF_IDH + 128]
            ident = CI[:]


            # ---------------- x0 transpose + HAM warm-up ----------------
            with tc.tile_pool(name="x0ps", bufs=2,
                              space=bass.MemorySpace.PSUM) as x0ps:
                # dummy matmuls on CI keep the PE active while the big const
                # loads stream in, so the HAM clock-gate opens (K=8/8) before
                # the first real encoder matmul instead of ~3.4us after
                warm = x0ps.tile([P, 512], f32, tag="x0", name="warm")
                for w in range(10):
                    nc.tensor.matmul(warm[:, 0:128], CI[:], CI[:],
                                     start=True, stop=True)
                for bt in range(2):
                    x0l = ld.tile([P, 16], f32, tag="x0l")
                    nc.scalar.dma_start(x0l[:], xs_ap[bt * 128:(bt + 1) * 128, 0, :])
                    tp = x0ps.tile([P, 512], f32, tag="x0")
                    nc.tensor.transpose(tp[0:16, 0:128], x0l[:], ident)
                    nc.vector.tensor_copy(x0T[:, bt * 128:(bt + 1) * 128],
                                          tp[0:16, 0:128])

            # ---------------- chain helper ----------------
            def chain_step(c, pool):
                # p8[c+1] = A8 p8[c] + sum_j K^{7-j} Bw u_{8c+j}  (s=8 col block)
                wp = pool.tile([P, 512], f32, tag="l2", name=f"wp{c}")
                nc.tensor.matmul(wp[0:80, 0:BC], cwh(0, 128, OFF_RUH + 560, OFF_RUH + 640),
                                 uT[:, c * BC:(c + 1) * BC], start=True, stop=False)
                if c == 0:
                    nc.tensor.matmul(wp[0:80, 0:BC], cwr(0, 16, OFF_RX + 560, OFF_RX + 640),
                                     x0T[:], start=False, stop=False)
                    for kc in range(2):
                        nc.tensor.matmul(wp[0:80, 0:BC],
                                         cwr(0, 128, OFF_RG + kc * 640 + 560, OFF_RG + kc * 640 + 640),
                                         h2_0[:, kc, :], start=False, stop=(kc == 1))
                else:
                    nc.tensor.matmul(wp[0:80, 0:BC], cwr(0, 80, OFF_RA + 560, OFF_RA + 640),
                                     p8[:, c * BC:(c + 1) * BC], start=False, stop=True)
                pool_copy(p8[:, (c + 1) * BC:(c + 2) * BC], wp[0:80, 0:BC])

            # ---------------- phase C helpers ----------------
            # one group = (chunk c, batch half bt): 8 output steps.
            # Groups 0..15 run interleaved in sbs 8..15 as two [P,512] psum
            # halves (one bank), so their pred DMA streams during the encoder;
            # groups 16..31 run after the loop with a 4x[P,2,512] pool.
            pc_state = [0]
            pch_state = {"H": 0, "ob": None}

            def pc_mms(dst320, c, bt, hh):
                bsl = slice(bt * 128, (bt + 1) * 128)
                if c == 0:
                    nc.tensor.matmul(dst320, x0T[:, bsl],
                                     cwr(0, 16, OFF_RX + hh * 320, OFF_RX + (hh + 1) * 320),
                                     start=True, stop=False)
                    for kc in range(2):
                        nc.tensor.matmul(dst320, h2_0[:, kc, bsl],
                                         cwr(0, 128, OFF_RG + kc * 640 + hh * 320,
                                             OFF_RG + kc * 640 + (hh + 1) * 320),
                                         start=False, stop=False)
                else:
                    nc.tensor.matmul(dst320,
                                     p8[:, c * BC + bt * 128:c * BC + (bt + 1) * 128],
                                     cwr(0, 80, OFF_RA + hh * 320, OFF_RA + (hh + 1) * 320),
                                     start=True, stop=False)
                nc.tensor.matmul(dst320,
                                 uT[:, c * BC + bt * 128:c * BC + (bt + 1) * 128],
                                 cwh(0, 128, OFF_RUH + hh * 320, OFF_RUH + (hh + 1) * 320),
                                 start=False, stop=True)

            def pc_half(pool):
                st = pch_state
                H = st["H"]
                if H >= 32:
                    return
                st["H"] = H + 1
                G = H // 2
                hh = H % 2
                c, bt = G // 2, G % 2
                if hh == 0:
                    ob = osbp.tile([P, 640], f32, tag="ob", name=f"obh{H}")
                    st["ob"] = ob
                ob = st["ob"]
                op = pool.tile([P, 512], f32, tag="pc", name=f"pch{H}")
                pc_mms(op[:, 0:320], c, bt, hh)
                if hh == 0:
                    nc.vector.tensor_copy(ob[:, 0:320], op[:, 0:320])
                else:
                    nc.scalar.copy(ob[:, 320:640], op[:, 0:320])
                    nsteps = S if c < NCH - 1 else S - 1
                    nc.sync.dma_start(
                        pred_d[bt * 128:(bt + 1) * 128, c * S:c * S + nsteps, :],
                        ob[:, 0:nsteps * L].rearrange("p (s l) -> p s l", l=L))

            def pc_group(pool):
                G = pc_state[0]
                if G >= 2 * NCH:
                    return
                pc_state[0] += 1
                c, bt = G // 2, G % 2
                op = pool.tile([P, 2, 512], f32, tag="oC", name=f"op{G}")
                for hh in range(2):
                    pc_mms(op[:, hh, 0:320], c, bt, hh)
                ob = osbp.tile([P, 640], f32, tag="ob")
                # split the psum->SBUF copy across both engines: halves the
                # per-group latency so the phase C pipeline stays dense
                nc.vector.tensor_copy(ob[:, (G % 2) * 320:(G % 2) * 320 + 320],
                                      op[:, G % 2, 0:320])
                nc.scalar.copy(ob[:, (1 - G % 2) * 320:(1 - G % 2) * 320 + 320],
                               op[:, 1 - G % 2, 0:320])
                eng = nc.sync if G % 2 == 0 else nc.scalar
                nsteps = S if c < NCH - 1 else S - 1
                eng.dma_start(
                    pred_d[bt * 128:(bt + 1) * 128, c * S:c * S + nsteps, :],
                    ob[:, 0:nsteps * L].rearrange("p (s l) -> p s l", l=L))



            with (
                tc.tile_pool(name="mixb", bufs=1, space=bass.MemorySpace.PSUM) as mixb,
                tc.tile_pool(name="l3ps", bufs=2, space=bass.MemorySpace.PSUM) as l3ps,
                tc.tile_pool(name="h1ps", bufs=2, space=bass.MemorySpace.PSUM) as h1ps,
                tc.tile_pool(name="l2ps", bufs=2, space=bass.MemorySpace.PSUM) as l2ps,
                tc.tile_pool(name="pcps", bufs=1, space=bass.MemorySpace.PSUM) as pcps,
            ):
                # ---------------- encoder superblocks ----------------
                for sb in range(NSB):
                    # x gather: per bt-half, each partition's 8 (or 9)
                    # timesteps are one contiguous 576B run -> 128 descriptors
                    # per DMA (descriptor generation on Sync is the scarce
                    # resource). The PE transposes then read overlapping
                    # 32-col windows (16 real + 16 zero-weighted pad) via a
                    # custom AP. ax[p, bt*144 + j] = xs[bt*128+p, 8sb + j//16, j%16]
                    ax = ld.tile([P, 288], f32, tag="ax")
                    last_sb = sb == NSB - 1
                    nld = 128 if last_sb else 144
                    ax_eng = nc.gpsimd if sb == 0 else nc.sync
                    for bt in range(2):
                        in_ap = bass.AP(xs_d, bt * 128 * 2048 + 8 * sb * 16,
                                        [[2048, 128], [1, nld]])
                        ax_eng.dma_start(ax[:, bt * 144:bt * 144 + nld], in_ap)
                    if last_sb:
                        # cols 272:288 of this rotating buffer are stale (the
                        # 128-elem load stops at t=127); overwrite with finite
                        # data so the zero-weighted pad cannot be NaN garbage
                        nc.gpsimd.tensor_copy(ax[:, 272:288], ax[:, 0:16])
                    # one DVE pass builds the 4x(tt,g0) overlapping 32-col
                    # windows in bf16 (the PE rejects overlapping-window
                    # stationary APs; the DVE does not care)
                    axp = ax[:]
                    axw = xTp.tile([P, 512], bf16, tag="axw")
                    srcw = bass.AP(axp.tensor, axp.offset,
                                   [[axp.ap[0][0], 128], [32, 4], [16, 2],
                                    [144, 2], [1, 32]])
                    nc.gpsimd.tensor_copy(
                        axw[:].rearrange("p (a b c d) -> p a b c d",
                                         a=4, b=2, c=2), srcw)
                    xts = xTp.tile([P, 512], bf16, tag="xT")
                    tpx = mixb.tile([P, 512], bf16, tag="mixb")
                    for i in range(4):
                        nc.tensor.transpose(tpx[:, i * 128:(i + 1) * 128],
                                            axw[:, i * 128:(i + 1) * 128],
                                            identh)
                    pool_copy(xts[:], tpx[:],
                              alt=nc.vector if sb % 2 == 0 else nc.scalar)

                    # u loads for sb<8: gpsimd cast-DMA (f32->bf16); PE
                    # transposes (1-pass bf16) emitted via maybe_u below
                    au = None
                    utp_ref = [None]
                    if sb < 8:
                        au = aupool.tile([P, 2, 256], bf16, tag="au")
                        for bt in range(2):
                            nc.gpsimd.dma_start(
                                au[:, bt, :],
                                us2d[bt * 128:(bt + 1) * 128,
                                     2 * sb * 128:(2 * sb + 2) * 128])
                    if sb == 0:
                        # exact-W0/W1 block (~0.4MB): first use is h2_0 at sb1
                        nc.sync.dma_start(CW[:, OFF_W0R:CW_COLS],
                                          cw_d[:, OFF_W0R:CW_COLS])
                    elif sb == 1:
                        # recurrence consts (~1.3MB) split across both HWDGE
                        # queues; first use is the chain at sb4 / phase C
                        nc.sync.dma_start(CW[:, 0:1280], cw_d[:, 0:1280])
                        nc.scalar.dma_start(CW[:, 1280:OFF_W0R],
                                            cw_d[:, 1280:OFF_W0R])

                    h1s = [h1sbp.tile([P, 2048], bf16, tag=f"h1sb{mc}", name=f"h1s{mc}")
                           for mc in range(2)]
                    h2s = [h2sbp.tile([P, 2048], bf16, tag=f"h2sb{mc}", name=f"h2s{mc}")
                           for mc in range(2)]
                    gps = [l3ps.tile([P, 512], f32, tag="l3", name=f"gps{h}")
                           for h in range(2)]

                    def emit_l1(g):
                        for mc in range(2):
                            hp = h1ps.tile([P, 512], f32, tag="h1", name="hp")
                            nc.tensor.matmul(hp[:],
                                             cwh(g * 32, (g + 1) * 32, OFF_W0 + mc * 128, OFF_W0 + (mc + 1) * 128),
                                             xts[g * 32:(g + 1) * 32, :],
                                             start=True, stop=True,
                                             tile_position=(32 * g, 0))
                            if mc == 0:
                                nc.scalar.activation(
                                    h1s[mc][:, g * 512:(g + 1) * 512], hp[:],
                                    AF.Relu, bias=CB[:, mc:mc + 1])
                            else:
                                nc.vector.tensor_scalar(
                                    h1s[mc][:, g * 512:(g + 1) * 512], hp[:],
                                    CB[:, mc:mc + 1], 0.0,
                                    op0=ALU.add, op1=ALU.max)

                    def emit_l2(nb):
                        for mc in range(2):
                            lp = l2ps.tile([P, 512], f32, tag="l2", name="lp")
                            for kc in range(2):
                                nc.tensor.matmul(lp[:],
                                                 cwh(0, 128, OFF_W1 + kc * 256 + mc * 128,
                                                     OFF_W1 + kc * 256 + (mc + 1) * 128),
                                                 h1s[kc][:, nb * 512:(nb + 1) * 512],
                                                 start=(kc == 0), stop=(kc == 1))
                            if mc == 0:
                                nc.vector.tensor_scalar(
                                    h2s[mc][:, nb * 512:(nb + 1) * 512], lp[:],
                                    CB[:, 2 + mc:3 + mc], 0.0,
                                    op0=ALU.add, op1=ALU.max)
                            else:
                                nc.scalar.activation(
                                    h2s[mc][:, nb * 512:(nb + 1) * 512], lp[:],
                                    AF.Relu, bias=CB[:, 2 + mc:3 + mc])

                    def emit_l3(nb):
                        # flipped: out[token, gfeat] directly (no transpose)
                        h = nb % 2
                        for i in range(4):
                            tq = 2 * i + nb // 2
                            bk = nb * 4 + i
                            for kc in range(2):
                                nc.tensor.matmul(
                                    gps[h][:, tq * 64:(tq + 1) * 64],
                                    h2s[kc][:, bk * 128:(bk + 1) * 128],
                                    cwh(0, 128, OFF_W2 + kc * 64, OFF_W2 + (kc + 1) * 64),
                                    start=(kc == 0), stop=(kc == 1))

                    t_lo = 1 if sb == 0 else 0

                    def emit_out(h):
                        # assemble full 80-col rows: x-part from the compact ax
                        # tile (SBUF->SBUF, Pool engine), g-part from flipped-L3
                        # psum (DVE/ACT). One contiguous 2.5KB/partition DMA
                        # per half (128 descriptors).
                        gout = osbp.tile([P, 8, L], f32, tag=f"gout{h}")
                        nc.gpsimd.tensor_copy(
                            gout[:, t_lo:8, 0:16],
                            ax[:, h * 144 + 16 * t_lo:h * 144 + 128].rearrange(
                                "p (t d) -> p t d", d=16))
                        pool_copy(gout[:, :, 16:80],
                                  gps[h][:].rearrange("p (t l) -> p t l", l=64),
                                  alt=nc.vector if h == 0 else nc.scalar)
                        (nc.sync if h == 0 else nc.scalar).dma_start(
                            y2s_d[h * 128:(h + 1) * 128,
                                  8 * sb - 1 + t_lo:8 * sb + 7, :],
                            gout[:, t_lo:8, :])

                    def maybe_u(j):
                        # j in {0,1}: two bf16 transposes each into utp; after
                        # j==1, one merged DVE copy into uT (512 bf16 cols)
                        if au is None:
                            return
                        if j == 0:
                            utp_ref[0] = l2ps.tile([P, 512], bf16, tag="l2",
                                                   name=f"utp{sb}")
                        utp = utp_ref[0]
                        for cl_bt in (2 * j, 2 * j + 1):
                            cl, bt = cl_bt // 2, cl_bt % 2
                            nc.tensor.transpose(
                                utp[:, cl_bt * 128:(cl_bt + 1) * 128],
                                au[:, bt, cl * 128:(cl + 1) * 128], identh)
                        if j == 1:
                            nc.vector.tensor_copy(
                                uT[:, 2 * sb * BC:(2 * sb + 2) * BC], utp[:])

                    def maybe_chain(j):
                        # chain step c emitted at sb 2..9, 2 per sb
                        if 2 <= sb <= 9:
                            c = 2 * (sb - 2) + j
                            if c < NCH - 1:
                                chain_step(c, l2ps)

                    def maybe_pc():
                        # sbs 8..15: two pc halves per slot point -> 2 groups/sb
                        if sb >= 8:
                            pc_half(pcps)

                    if sb == 1:
                        # h1_0 / h2_0 exact recompute (f32r) for chain/phase C;
                        # here so the 2MB CW load has drained off the queues
                        for mc in range(2):
                            hp = h1ps.tile([P, 512], f32, tag="h1",
                                           name=f"h10ps{mc}")
                            nc.tensor.matmul(
                                hp[:, 0:256],
                                cwr(0, 16, OFF_W0R + mc * 128, OFF_W0R + (mc + 1) * 128),
                                x0T[:], start=True, stop=True)
                            nc.scalar.activation(h10[:, mc, :], hp[:, 0:256],
                                                 AF.Relu, bias=CB[:, mc:mc + 1])
                        for mc in range(2):
                            lp = l2ps.tile([P, 512], f32, tag="l2",
                                           name=f"h20ps{mc}")
                            for kc in range(2):
                                nc.tensor.matmul(
                                    lp[:, 0:256],
                                    cwr(0, 128, OFF_W1R + kc * 256 + mc * 128,
                                        OFF_W1R + kc * 256 + (mc + 1) * 128),
                                    h10[:, kc, :], start=(kc == 0), stop=(kc == 1))
                            nc.vector.tensor_scalar(h2_0[:, mc, :], lp[:, 0:256],
                                                    CB[:, 2 + mc:3 + mc], 0.0,
                                                    op0=ALU.add, op1=ALU.max)

                    emit_l1(0)
                    emit_l1(1)
                    maybe_u(0)
                    emit_l1(2)
                    emit_l2(0)
                    maybe_chain(0)
                    maybe_pc()
                    emit_l1(3)
                    emit_l2(1)
                    emit_l3(0)
                    maybe_u(1)
                    emit_l2(2)
                    maybe_pc()
                    emit_l3(1)
                    emit_l2(3)
                    emit_l3(2)
                    emit_out(0)
                    maybe_chain(1)
                    maybe_pc()
                    emit_l3(3)
                    emit_out(1)
                    maybe_pc()

            if dbg_d is not None:
                nc.sync.dma_start(dbg_d[:], uT[:])

            # ---------------- phase C: remaining groups, dedicated 16KB pool ----
            pc_state[0] = pch_state["H"] // 2
            with tc.tile_pool(name="oC", bufs=4, space=bass.MemorySpace.PSUM) as oC:
                for _ in range(2 * NCH - pc_state[0]):
                    pc_group(oC)

    nc.compile()
    return nc


def kernel(xs, us, W0, b0, W1, b1, W2, Bw, K):
    global _BUILT
    from concourse.bass_utils import run_bass_kernel_spmd

    if _BUILT is None:
        _BUILT = _build()
    nc = _BUILT

    CW, CWH, CB, CI = _precompute_consts(W0, b0, W1, b1, W2, Bw, K)
    xs = np.ascontiguousarray(np.asarray(xs, np.float32))
    us = np.ascontiguousarray(np.asarray(us, np.float32))

    in_maps = []
    for k in range(NCORES):
        sl = slice(k * BC, (k + 1) * BC)
        in_maps.append({"xs": xs[sl], "us": us[sl], "CW": CW, "CWH": CWH,
                        "CB": CB, "CI": CI})

    res = run_bass_kernel_spmd(nc, in_maps, list(range(NCORES)),
                           trace=bool(int(os.environ.get("DEINA_TRACE", "0"))))
    y2s = np.concatenate([res.results[k]["y2s"] for k in range(NCORES)], axis=0)
    pred = np.concatenate([res.results[k]["y2s_pred"] for k in range(NCORES)], axis=0)
    kernel.last_exec_time_ns = res.exec_time_ns
    kernel.last_results = res
    return y2s, pred



# revision 62
# speedup vs baseline: 1.1499x; 1.1499x over previous
"""Trainium2 Bass kernel for the Koopman DEINA model.

Computation (per reference):
  encoder: h1 = relu(x W0^T + b0); h2 = relu(h1 W1^T + b1); g = h2 W2^T
  y2s      = concat(xs[:,1:], g[:,1:])                       [B,127,80]
  y2s_pred = scan: p_{n+1} = K p_n + Bw u_n, p_0 = [x_0; g_0], output p_1..p_127

Strategy (8 NeuronCores, data-parallel over batch, 256 batch elems/core):
  - Encoder in bf16 with fp32 psum; tokens processed t-major (token = t*256+b),
    x PE-transposed in 512-token tiles (4 groups x 32 cols, 16 zero-weight pad).
  - PE pipeline continuity is the design driver: matmuls are emitted so the
    array never waits on a drain (measured: back-to-back matmuls stream at
    ~0.42ns/row with weight loads hidden; dependency breaks cost ~2x).
  - y2s x-part: two whole-tensor DRAM->DRAM DMAs (no SBUF hop, no copies).
  - y2s g-part: f32 PE transposes into PSUM, DMA'd straight from PSUM.
  - pred: chunked closed form (S=8), phase-C outputs DMA'd straight from PSUM.
  - u transposes and the 15-step boundary chain are interleaved into the
    encoder superblock loop so their latency hides under encoder matmuls.
"""
import os
import numpy as np

DIM = 16
H1 = H2 = 256
G = 64
L = 80          # DIM + G
T = 128
NB = 2048       # full batch
NCORES = 8
BC = NB // NCORES   # 256 batch elems per core
S = 8           # recurrence chunk size
NCH = T // S    # 16 chunks
NTOK = BC * T   # 32768 tokens per core
NSB = NTOK // 2048   # 16 superblocks of 2048 tokens (= 8 t-steps x 256 b)

# f32r const tile column offsets (recurrence)
OFF_RA = 0        # [80, 640]   R_A[k,(s-1)*80+l] = (K^s)[l,k]
OFF_RX = 640      # [16, 640]   R_X[d,(s-1)*80+l] = (K^s)[l,d]
OFF_RG = 1280     # [128, 1280] R_G[kp, kc*640+(s-1)*80+l] = (K^s[:,16:] @ W2)[l, kc*128+kp]
OFF_W0R = 2560    # [16, 256]   exact W0T (f32r) for the h2_0 recompute
OFF_W1R = 2816    # [128, 512]  exact W1T (f32r)
CW_COLS = 3328
# bf16 const tile column offsets (encoder)
OFF_W0 = 0        # [128, 256]  W0T on 4x32-row groups (rows 0:16 of each)
OFF_W0S = 256     # [128, 256]  shifted variant (rows 16:32 of each group)
OFF_W1 = 512      # [128, 512]  W1T, kc blocks of 256 cols
OFF_W2 = 1024     # [128, 128]  W2T, kc blocks of 64 cols
OFF_IDH = 1152    # [128, 128]  bf16 identity (host-provided)
OFF_RUH = 1280    # [128, 640]  R_U[j*16+d,(s-1)*80+l] = (K^{s-1-j} Bw)[l,d], j<s (bf16)
CWH_COLS = 1920

_BUILT = None


def _precompute_consts(W0, b0, W1, b1, W2, Bw, K):
    """Host-side weight folding: CW [128,CW_COLS] f32(r), CWH [128,CWH_COLS] bf16,
    CB [128,4] f32."""
    import ml_dtypes
    W0 = np.asarray(W0, np.float64)
    W1 = np.asarray(W1, np.float64)
    W2 = np.asarray(W2, np.float64)
    Bw = np.asarray(Bw, np.float64)
    K = np.asarray(K, np.float64)

    CWH = np.zeros((128, CWH_COLS), np.float64)
    for g in range(4):
        CWH[g * 32:g * 32 + 16, OFF_W0:OFF_W0 + 256] = W0.T
        CWH[g * 32 + 16:g * 32 + 32, OFF_W0S:OFF_W0S + 256] = W0.T
    for kc in range(2):
        CWH[:, OFF_W1 + kc * 256:OFF_W1 + (kc + 1) * 256] = W1[:, kc * 128:(kc + 1) * 128].T
        CWH[:, OFF_W2 + kc * 64:OFF_W2 + (kc + 1) * 64] = W2[:, kc * 128:(kc + 1) * 128].T

    CW = np.zeros((128, CW_COLS), np.float64)
    CW[0:16, OFF_W0R:OFF_W0R + 256] = W0.T
    for kc in range(2):
        CW[:, OFF_W1R + kc * 256:OFF_W1R + (kc + 1) * 256] = W1[:, kc * 128:(kc + 1) * 128].T
    A = [np.eye(L)]
    for _ in range(S):
        A.append(K @ A[-1])
    Dm = [Bw]
    for _ in range(S - 1):
        Dm.append(K @ Dm[-1])
    for s in range(1, S + 1):
        col = (s - 1) * L
        CW[0:L, OFF_RA + col:OFF_RA + col + L] = A[s].T
        CW[0:16, OFF_RX + col:OFF_RX + col + L] = A[s][:, :16].T
        AsG = A[s][:, 16:] @ W2          # [80, 256]
        for kc in range(2):
            CW[:, OFF_RG + kc * 640 + col:OFF_RG + kc * 640 + col + L] = \
                AsG[:, kc * 128:(kc + 1) * 128].T
        for j in range(s):
            CWH[j * 16:(j + 1) * 16, OFF_RUH + col:OFF_RUH + col + L] = Dm[s - 1 - j].T

    CWH[:, OFF_IDH:OFF_IDH + 128] = np.eye(128)

    CB = np.zeros((128, 4), np.float64)
    CB[:, 0] = np.asarray(b0)[0:128]
    CB[:, 1] = np.asarray(b0)[128:256]
    CB[:, 2] = np.asarray(b1)[0:128]
    CB[:, 3] = np.asarray(b1)[128:256]
    return (CW.astype(np.float32), CWH.astype(ml_dtypes.bfloat16),
            CB.astype(np.float32), np.eye(128, dtype=np.float32))


def _build():
    import concourse.bass as bass
    import concourse.bacc as bacc
    import concourse.mybir as mybir
    import concourse.tile as tile
    
    f32 = mybir.dt.float32
    f32r = mybir.dt.float32r
    bf16 = mybir.dt.bfloat16
    AF = mybir.ActivationFunctionType
    ALU = mybir.AluOpType
    P = 128

    nc = bacc.Bacc(None, target_bir_lowering=False, debug=False)

    xs_d = nc.dram_tensor("xs", [BC, T, DIM], f32, kind="ExternalInput")
    us_d = nc.dram_tensor("us", [BC, T, DIM], f32, kind="ExternalInput")
    cw_d = nc.dram_tensor("CW", [P, CW_COLS], f32r, kind="ExternalInput")
    cwh_d = nc.dram_tensor("CWH", [P, CWH_COLS], bf16, kind="ExternalInput")
    cb_d = nc.dram_tensor("CB", [P, 4], f32, kind="ExternalInput")
    ci_d = nc.dram_tensor("CI", [P, P], f32, kind="ExternalInput")
    y2s_d = nc.dram_tensor("y2s", [BC, T - 1, L], f32, kind="ExternalOutput")
    pred_d = nc.dram_tensor("y2s_pred", [BC, T - 1, L], f32, kind="ExternalOutput")
    dbg_d = None
    if os.environ.get("DEINA_DBG_UT"):
        dbg_d = nc.dram_tensor("dbg_uT", [P, NCH * BC], bf16, kind="ExternalOutput")

    xs_ap = xs_d.ap()
    us2d = us_d.ap().rearrange("b t d -> b (t d)")     # [256, 2048]

    with tile.TileContext(nc) as tc:
        with (
            tc.tile_pool(name="consts", bufs=1) as consts,
            tc.tile_pool(name="store", bufs=1) as store,
            tc.tile_pool(name="ld", bufs=4) as ld,
            tc.tile_pool(name="aupool", bufs=8) as aupool,
            tc.tile_pool(name="xT", bufs=3) as xTp,
            tc.tile_pool(name="h1sb", bufs=2) as h1sbp,
            tc.tile_pool(name="h2sb", bufs=2) as h2sbp,
            tc.tile_pool(name="osb", bufs=8) as osbp,
        ):
            CI = consts.tile([P, P], f32)
            nc.sync.dma_start(CI[:], ci_d[:])
            CB = consts.tile([P, 4], f32)
            nc.sync.dma_start(CB[:], cb_d[:])
            CWH = consts.tile([P, CWH_COLS], bf16)
            nc.sync.dma_start(CWH[:], cwh_d[:])
            CW = consts.tile([P, CW_COLS], f32r)

            uT = store.tile([P, NCH * BC], bf16)       # [(j,d), c*256 + b]
            p8 = store.tile([80, NCH * BC], f32r)      # block c = p_{8c} (c>=1)
            x0T = store.tile([16, BC], f32r)
            h10 = store.tile([P, 2, BC], f32r)
            h2_0 = store.tile([P, 2, BC], f32r)


            def pool_copy(dst, src, alt=None):
                # psum-sourced copy: Pool/GpSimd cannot read PSUM, so these
                # go to DVE (vector) or ACT (scalar)
                e = alt or nc.vector
                if e is nc.scalar:
                    e.copy(dst, src)
                else:
                    e.tensor_copy(dst, src)

            def cwr(r0, r1, c0, c1):
                return CW[r0:r1, c0:c1]

            def cwh(r0, r1, c0, c1):
                return CWH[r0:r1, c0:c1]

            identh = CWH[:, OFF_IDH:OFF_IDH + 128]
            ident = CI[:]


            # ---------------- x0 transpose + HAM warm-up ----------------
            with tc.tile_pool(name="x0ps", bufs=2,
                              space=bass.MemorySpace.PSUM) as x0ps:
                # dummy matmuls on CI keep the PE active while the big const
                # loads stream in, so the HAM clock-gate opens (K=8/8) before
                # the first real encoder matmul instead of ~3.4us after
                warm = x0ps.tile([P, 512], f32, tag="x0", name="warm")
                for w in range(10):
                    nc.tensor.matmul(warm[:, 0:128], CI[:], CI[:],
                                     start=True, stop=True)
                for bt in range(2):
                    x0l = ld.tile([P, 16], f32, tag="x0l")
                    nc.scalar.dma_start(x0l[:], xs_ap[bt * 128:(bt + 1) * 128, 0, :])
                    tp = x0ps.tile([P, 512], f32, tag="x0")
                    nc.tensor.transpose(tp[0:16, 0:128], x0l[:], ident)
                    nc.vector.tensor_copy(x0T[:, bt * 128:(bt + 1) * 128],
                                          tp[0:16, 0:128])

            # ---------------- chain helper ----------------
            def chain_step(c, pool):
                # p8[c+1] = A8 p8[c] + sum_j K^{7-j} Bw u_{8c+j}  (s=8 col block)
                wp = pool.tile([P, 512], f32, tag="l2", name=f"wp{c}")
                nc.tensor.matmul(wp[0:80, 0:BC], cwh(0, 128, OFF_RUH + 560, OFF_RUH + 640),
                                 uT[:, c * BC:(c + 1) * BC], start=True, stop=False)
                if c == 0:
                    nc.tensor.matmul(wp[0:80, 0:BC], cwr(0, 16, OFF_RX + 560, OFF_RX + 640),
                                     x0T[:], start=False, stop=False)
                    for kc in range(2):
                        nc.tensor.matmul(wp[0:80, 0:BC],
                                         cwr(0, 128, OFF_RG + kc * 640 + 560, OFF_RG + kc * 640 + 640),
                                         h2_0[:, kc, :], start=False, stop=(kc == 1))
                else:
                    nc.tensor.matmul(wp[0:80, 0:BC], cwr(0, 80, OFF_RA + 560, OFF_RA + 640),
                                     p8[:, c * BC:(c + 1) * BC], start=False, stop=True)
                pool_copy(p8[:, (c + 1) * BC:(c + 2) * BC], wp[0:80, 0:BC])

            # ---------------- phase C helpers ----------------
            # one group = (chunk c, batch half bt): 8 output steps.
            # Groups 0..15 run interleaved in sbs 8..15 as two [P,512] psum
            # halves (one bank), so their pred DMA streams during the encoder;
            # groups 16..31 run after the loop with a 4x[P,2,512] pool.
            pc_state = [0]
            pch_state = {"H": 0, "ob": None}

            def pc_mms(dst320, c, bt, hh):
                bsl = slice(bt * 128, (bt + 1) * 128)
                if c == 0:
                    nc.tensor.matmul(dst320, x0T[:, bsl],
                                     cwr(0, 16, OFF_RX + hh * 320, OFF_RX + (hh + 1) * 320),
                                     start=True, stop=False)
                    for kc in range(2):
                        nc.tensor.matmul(dst320, h2_0[:, kc, bsl],
                                         cwr(0, 128, OFF_RG + kc * 640 + hh * 320,
                                             OFF_RG + kc * 640 + (hh + 1) * 320),
                                         start=False, stop=False)
                else:
                    nc.tensor.matmul(dst320,
                                     p8[:, c * BC + bt * 128:c * BC + (bt + 1) * 128],
                                     cwr(0, 80, OFF_RA + hh * 320, OFF_RA + (hh + 1) * 320),
                                     start=True, stop=False)
                nc.tensor.matmul(dst320,
                                 uT[:, c * BC + bt * 128:c * BC + (bt + 1) * 128],
                                 cwh(0, 128, OFF_RUH + hh * 320, OFF_RUH + (hh + 1) * 320),
                                 start=False, stop=True)

            def pc_half(pool):
                st = pch_state
                H = st["H"]
                if H >= 32:
                    return
                st["H"] = H + 1
                G = H // 2
                hh = H % 2
                c, bt = G // 2, G % 2
                if hh == 0:
                    ob = osbp.tile([P, 640], f32, tag="ob", name=f"obh{H}")
                    st["ob"] = ob
                ob = st["ob"]
                op = pool.tile([P, 512], f32, tag="pc", name=f"pch{H}")
                pc_mms(op[:, 0:320], c, bt, hh)
                if hh == 0:
                    nc.vector.tensor_copy(ob[:, 0:320], op[:, 0:320])
                else:
                    nc.scalar.copy(ob[:, 320:640], op[:, 0:320])
                    nsteps = S if c < NCH - 1 else S - 1
                    nc.sync.dma_start(
                        pred_d[bt * 128:(bt + 1) * 128, c * S:c * S + nsteps, :],
                        ob[:, 0:nsteps * L].rearrange("p (s l) -> p s l", l=L))

            def pc_group(pool):
                G = pc_state[0]
                if G >= 2 * NCH:
                    return
                pc_state[0] += 1
                c, bt = G // 2, G % 2
                op = pool.tile([P, 2, 512], f32, tag="oC", name=f"op{G}")
                for hh in range(2):
                    pc_mms(op[:, hh, 0:320], c, bt, hh)
                ob = osbp.tile([P, 640], f32, tag="ob")
                if G % 2 == 0:
                    nc.vector.tensor_copy(
                        ob[:].rearrange("p (a b) -> p a b", a=2), op[:, :, 0:320])
                else:
                    nc.scalar.copy(
                        ob[:].rearrange("p (a b) -> p a b", a=2), op[:, :, 0:320])
                eng = nc.sync if G % 2 == 0 else nc.scalar
                nsteps = S if c < NCH - 1 else S - 1
                eng.dma_start(
                    pred_d[bt * 128:(bt + 1) * 128, c * S:c * S + nsteps, :],
                    ob[:, 0:nsteps * L].rearrange("p (s l) -> p s l", l=L))



            with (
                tc.tile_pool(name="mixb", bufs=2, space=bass.MemorySpace.PSUM) as mixb,
                tc.tile_pool(name="l3ps", bufs=2, space=bass.MemorySpace.PSUM) as l3ps,
                tc.tile_pool(name="h1ps", bufs=2, space=bass.MemorySpace.PSUM) as h1ps,
                tc.tile_pool(name="l2ps", bufs=2, space=bass.MemorySpace.PSUM) as l2ps,
            ):
                # ---------------- encoder superblocks ----------------
                for sb in range(NSB):
                    # x gather: per bt-half, each partition's 8 (or 9)
                    # timesteps are one contiguous 576B run -> 128 descriptors
                    # per DMA (descriptor generation on Sync is the scarce
                    # resource). The PE transposes then read overlapping
                    # 32-col windows (16 real + 16 zero-weighted pad) via a
                    # custom AP. ax[p, bt*144 + j] = xs[bt*128+p, 8sb + j//16, j%16]
                    ax = ld.tile([P, 288], f32, tag="ax")
                    last_sb = sb == NSB - 1
                    nld = 128 if last_sb else 144
                    ax_eng = nc.gpsimd if sb == 0 else nc.sync
                    for bt in range(2):
                        in_ap = bass.AP(xs_d, bt * 128 * 2048 + 8 * sb * 16,
                                        [[2048, 128], [1, nld]])
                        ax_eng.dma_start(ax[:, bt * 144:bt * 144 + nld], in_ap)
                    if last_sb:
                        # cols 272:288 of this rotating buffer are stale (the
                        # 128-elem load stops at t=127); overwrite with finite
                        # data so the zero-weighted pad cannot be NaN garbage
                        nc.gpsimd.tensor_copy(ax[:, 272:288], ax[:, 0:16])
                    # one DVE pass builds the 4x(tt,g0) overlapping 32-col
                    # windows in bf16 (the PE rejects overlapping-window
                    # stationary APs; the DVE does not care)
                    axp = ax[:]
                    axw = xTp.tile([P, 512], bf16, tag="axw")
                    srcw = bass.AP(axp.tensor, axp.offset,
                                   [[axp.ap[0][0], 128], [32, 4], [16, 2],
                                    [144, 2], [1, 32]])
                    nc.gpsimd.tensor_copy(
                        axw[:].rearrange("p (a b c d) -> p a b c d",
                                         a=4, b=2, c=2), srcw)
                    xts = xTp.tile([P, 512], bf16, tag="xT")
                    tpx = mixb.tile([P, 512], bf16, tag="mixb")
                    for i in range(4):
                        nc.tensor.transpose(tpx[:, i * 128:(i + 1) * 128],
                                            axw[:, i * 128:(i + 1) * 128],
                                            identh)
                    pool_copy(xts[:], tpx[:],
                              alt=nc.vector if sb % 2 == 0 else nc.scalar)

                    # u loads for sb<8: gpsimd cast-DMA (f32->bf16); PE
                    # transposes (1-pass bf16) emitted via maybe_u below
                    au = None
                    utp_ref = [None]
                    if sb < 8:
                        au = aupool.tile([P, 2, 256], bf16, tag="au")
                        for bt in range(2):
                            nc.gpsimd.dma_start(
                                au[:, bt, :],
                                us2d[bt * 128:(bt + 1) * 128,
                                     2 * sb * 128:(2 * sb + 2) * 128])
                    if sb == 0:
                        # exact-W0/W1 block (~0.4MB): first use is h2_0 at sb1
                        nc.sync.dma_start(CW[:, OFF_W0R:CW_COLS],
                                          cw_d[:, OFF_W0R:CW_COLS])
                    elif sb == 1:
                        # recurrence consts (~1.3MB) split across both HWDGE
                        # queues; first use is the chain at sb4 / phase C
                        nc.sync.dma_start(CW[:, 0:1280], cw_d[:, 0:1280])
                        nc.scalar.dma_start(CW[:, 1280:OFF_W0R],
                                            cw_d[:, 1280:OFF_W0R])

                    h1s = [h1sbp.tile([P, 2048], bf16, tag=f"h1sb{mc}", name=f"h1s{mc}")
                           for mc in range(2)]
                    h2s = [h2sbp.tile([P, 2048], bf16, tag=f"h2sb{mc}", name=f"h2s{mc}")
                           for mc in range(2)]
                    gps = [l3ps.tile([P, 512], f32, tag="l3", name=f"gps{h}")
                           for h in range(2)]

                    def emit_l1(g):
                        for mc in range(2):
                            hp = h1ps.tile([P, 512], f32, tag="h1", name="hp")
                            nc.tensor.matmul(hp[:],
                                             cwh(g * 32, (g + 1) * 32, OFF_W0 + mc * 128, OFF_W0 + (mc + 1) * 128),
                                             xts[g * 32:(g + 1) * 32, :],
                                             start=True, stop=True,
                                             tile_position=(32 * g, 0))
                            if mc == 0:
                                nc.scalar.activation(
                                    h1s[mc][:, g * 512:(g + 1) * 512], hp[:],
                                    AF.Relu, bias=CB[:, mc:mc + 1])
                            else:
                                nc.vector.tensor_scalar(
                                    h1s[mc][:, g * 512:(g + 1) * 512], hp[:],
                                    CB[:, mc:mc + 1], 0.0,
                                    op0=ALU.add, op1=ALU.max)

                    def emit_l2(nb):
                        for mc in range(2):
                            lp = l2ps.tile([P, 512], f32, tag="l2", name="lp")
                            for kc in range(2):
                                nc.tensor.matmul(lp[:],
                                                 cwh(0, 128, OFF_W1 + kc * 256 + mc * 128,
                                                     OFF_W1 + kc * 256 + (mc + 1) * 128),
                                                 h1s[kc][:, nb * 512:(nb + 1) * 512],
                                                 start=(kc == 0), stop=(kc == 1))
                            if mc == 0:
                                nc.vector.tensor_scalar(
                                    h2s[mc][:, nb * 512:(nb + 1) * 512], lp[:],
                                    CB[:, 2 + mc:3 + mc], 0.0,
                                    op0=ALU.add, op1=ALU.max)
                            else:
                                nc.scalar.activation(
                                    h2s[mc][:, nb * 512:(nb + 1) * 512], lp[:],
                                    AF.Relu, bias=CB[:, 2 + mc:3 + mc])

                    def emit_l3(nb):
                        # flipped: out[token, gfeat] directly (no transpose)
                        h = nb % 2
                        for i in range(4):
                            tq = 2 * i + nb // 2
                            bk = nb * 4 + i
                            for kc in range(2):
                                nc.tensor.matmul(
                                    gps[h][:, tq * 64:(tq + 1) * 64],
                                    h2s[kc][:, bk * 128:(bk + 1) * 128],
                                    cwh(0, 128, OFF_W2 + kc * 64, OFF_W2 + (kc + 1) * 64),
                                    start=(kc == 0), stop=(kc == 1))

                    t_lo = 1 if sb == 0 else 0

                    def emit_out(h):
                        # assemble full 80-col rows: x-part from the compact ax
                        # tile (SBUF->SBUF, Pool engine), g-part from flipped-L3
                        # psum (DVE/ACT). One contiguous 2.5KB/partition DMA
                        # per half (128 descriptors).
                        gout = osbp.tile([P, 8, L], f32, tag=f"gout{h}")
                        nc.gpsimd.tensor_copy(
                            gout[:, t_lo:8, 0:16],
                            ax[:, h * 144 + 16 * t_lo:h * 144 + 128].rearrange(
                                "p (t d) -> p t d", d=16))
                        pool_copy(gout[:, :, 16:80],
                                  gps[h][:].rearrange("p (t l) -> p t l", l=64),
                                  alt=nc.vector if h == 0 else nc.scalar)
                        (nc.sync if h == 0 else nc.scalar).dma_start(
                            y2s_d[h * 128:(h + 1) * 128,
                                  8 * sb - 1 + t_lo:8 * sb + 7, :],
                            gout[:, t_lo:8, :])

                    def maybe_u(j):
                        # j in {0,1}: two bf16 transposes each into utp; after
                        # j==1, one merged DVE copy into uT (512 bf16 cols)
                        if au is None:
                            return
                        if j == 0:
                            utp_ref[0] = l2ps.tile([P, 512], bf16, tag="l2",
                                                   name=f"utp{sb}")
                        utp = utp_ref[0]
                        for cl_bt in (2 * j, 2 * j + 1):
                            cl, bt = cl_bt // 2, cl_bt % 2
                            nc.tensor.transpose(
                                utp[:, cl_bt * 128:(cl_bt + 1) * 128],
                                au[:, bt, cl * 128:(cl + 1) * 128], identh)
                        if j == 1:
                            nc.vector.tensor_copy(
                                uT[:, 2 * sb * BC:(2 * sb + 2) * BC], utp[:])

                    def maybe_chain(j):
                        # chain step c emitted at sb 4..11, 2 per sb
                        if 4 <= sb <= 11:
                            c = 2 * (sb - 4) + j
                            if c < NCH - 1:
                                chain_step(c, l2ps)

                    if sb == 1:
                        # h1_0 / h2_0 exact recompute (f32r) for chain/phase C;
                        # here so the 2MB CW load has drained off the queues
                        for mc in range(2):
                            hp = h1ps.tile([P, 512], f32, tag="h1",
                                           name=f"h10ps{mc}")
                            nc.tensor.matmul(
                                hp[:, 0:256],
                                cwr(0, 16, OFF_W0R + mc * 128, OFF_W0R + (mc + 1) * 128),
                                x0T[:], start=True, stop=True)
                            nc.scalar.activation(h10[:, mc, :], hp[:, 0:256],
                                                 AF.Relu, bias=CB[:, mc:mc + 1])
                        for mc in range(2):
                            lp = l2ps.tile([P, 512], f32, tag="l2",
                                           name=f"h20ps{mc}")
                            for kc in range(2):
                                nc.tensor.matmul(
                                    lp[:, 0:256],
                                    cwr(0, 128, OFF_W1R + kc * 256 + mc * 128,
                                        OFF_W1R + kc * 256 + (mc + 1) * 128),
                                    h10[:, kc, :], start=(kc == 0), stop=(kc == 1))
                            nc.vector.tensor_scalar(h2_0[:, mc, :], lp[:, 0:256],
                                                    CB[:, 2 + mc:3 + mc], 0.0,
                                                    op0=ALU.add, op1=ALU.max)

                    emit_l1(0)
                    emit_l1(1)
                    maybe_u(0)
                    emit_l1(2)
                    emit_l2(0)
                    maybe_chain(0)
                    emit_l1(3)
                    emit_l2(1)
                    emit_l3(0)
                    maybe_u(1)
                    emit_l2(2)
                    emit_l3(1)
                    emit_l2(3)
                    emit_l3(2)
                    emit_out(0)
                    maybe_chain(1)
                    emit_l3(3)
                    emit_out(1)

            if dbg_d is not None:
                nc.sync.dma_start(dbg_d[:], uT[:])

            # ---------------- phase C: remaining groups, dedicated 16KB pool ----
            pc_state[0] = pch_state["H"] // 2
            with tc.tile_pool(name="oC", bufs=4, space=bass.MemorySpace.PSUM) as oC:
                for _ in range(2 * NCH - pc_state[0]):
                    pc_group(oC)

    nc.compile()
    return nc


def kernel(xs, us, W0, b0, W1, b1, W2, Bw, K):
    global _BUILT
    from concourse.bass_utils import run_bass_kernel_spmd

    if _BUILT is None:
        _BUILT = _build()
    nc = _BUILT

    CW, CWH, CB, CI = _precompute_consts(W0, b0, W1, b1, W2, Bw, K)
    xs = np.ascontiguousarray(np.asarray(xs, np.float32))
    us = np.ascontiguousarray(np.asarray(us, np.float32))

    in_maps = []
    for k in range(NCORES):
        sl = slice(k * BC, (k + 1) * BC)
        in_maps.append({"xs": xs[sl], "us": us[sl], "CW": CW, "CWH": CWH,
                        "CB": CB, "CI": CI})

    res = run_bass_kernel_spmd(nc, in_maps, list(range(NCORES)),
                           trace=bool(int(os.environ.get("DEINA_TRACE", "0"))))
    y2s = np.concatenate([res.results[k]["y2s"] for k in range(NCORES)], axis=0)
    pred = np.concatenate([res.results[k]["y2s_pred"] for k in range(NCORES)], axis=0)
    kernel.last_exec_time_ns = res.exec_time_ns
    kernel.last_results = res
    return y2s, pred

